# revision 38
# baseline (speedup 1.0000x reference)
"""PatchSelfConv Trainium2 kernel.

Per-sample dynamic conv: each image (3,224,224) is cross-correlated
(stride 1, VALID) with its own 196 unfolded 16x16 patches as filters
-> out (196, 209, 209) per image. Batch 8 -> data-parallel, 1 image
per NeuronCore.

Per core this is out[n, y, x] = sum_{c,dy,dx} w[n,c,dy,dx] * img[c,y+dy,x+dx]
 = a [196, 768] @ [768, 209*209] matmul fed by an implicit im2col.

Implementation notes:
 - Contraction k=(c,dy,dx) is split as (c, dy, dxl+8*s): partitions hold
   (dy, dxl) with dxl in [0,8), and the remaining dx shift s in {0,1} is
   folded into the rhs access-pattern offset (+8s columns). This gives
   6 accumulating matmuls (3 c-tiles x 2 shifts) of K=128 per output
   row-pair per M-chunk, while the materialized rhs only replicates
   rows 8x instead of 16x.
 - rhs tiles [128=(dy,dxl), R*224] are built by a single 3-dim
   overlapping-window DMA per channel straight from the DRAM image
   (pre-cast to fp16 on host). Row windows are stored at width 224
   (full image rows, shifted by dxl) so (h,w) merge into one
   contiguous AP dim; matmuls read [2x209] windows at offset
   h*224 + 8*s + x.
 - Weights (the image's own patches, [768, 196] transposed-for-lhsT)
   are extracted on host (pure relayout of the input) and passed as a
   second per-core input tensor.
 - M=196 filters -> 2 chunks (128, 68). N=418 (2 output rows) per PSUM
   bank. PSUM -> SBUF via DVE cast-copy -> fp16 DMA to DRAM. Small
   6-row blocks keep the PE 100% dense (finer DMA/compute overlap).
"""
import numpy as np

C = 3
H = W = 224
P = 16
NPR = 14          # patches per row
NP = NPR * NPR    # 196
OH = OW = H - P + 1   # 209
BATCH = 8
N_CORES = 8

R_BLOCK = 6      # output rows per block
M_CHUNKS = (128, 68)


def _build_program(dt_name="float16", out_dt_name="float16", max_blocks=None):
    import concourse.bass as bass
    import concourse.tile as tile
    from concourse import bacc, mybir

    dt = getattr(mybir.dt, dt_name)
    out_dt = getattr(mybir.dt, out_dt_name)

    nc = bacc.Bacc("TRN2", target_bir_lowering=False, debug=False)
    x_d = nc.dram_tensor("xh", [C, H, W], dt, kind="ExternalInput")
    w_d = nc.dram_tensor("w", [6, 128, NP], dt, kind="ExternalInput")
    out_d = nc.dram_tensor("out", [NP, OH * OW], out_dt, kind="ExternalOutput")

    n_full = OH // R_BLOCK
    tail_rows = OH - n_full * R_BLOCK

    with tile.TileContext(nc) as tc:
        with (
            tc.tile_pool(name="wp", bufs=1) as wp,
            tc.tile_pool(name="rhs", bufs=4) as rhsp,
            tc.tile_pool(name="ost", bufs=4) as ostp,
            tc.tile_pool(name="ps", bufs=8, space="PSUM") as psp,
        ):
            # weights: [128, 6, 196], t = c*2 + s
            w_sb = wp.tile([128, 6, NP], dt)
            src = w_d[:, :, :]
            src.ap = mybir.VecI64Pair([[NP, 128], [128 * NP, 6], [1, NP]])
            src.offset = 0
            nc.scalar.dma_start(w_sb[:], src)


            def do_block(y0, rows):
                # rhs tiles per channel: [128=(dy*8+dxl), rows*224]
                wfree = rows * W
                rhs = []
                # last 7 window elements are never read by any matmul
                # (max read = (rows-1)*W + 8 + 208 = wfree - 8); skipping
                # them keeps the final block's reads inside the image.
                wdma = wfree - 7
                for c in range(C):
                    t = rhsp.tile([128, wfree], dt, tag=f"rhs{c}")
                    s = x_d[:, :, :]
                    s.ap = mybir.VecI64Pair([[W, 16], [1, 8], [1, wdma]])
                    s.offset = c * H * W + y0 * W
                    eng = nc.sync if c != 1 else nc.scalar
                    eng.dma_start(t[:, 0:wdma], s)
                    rhs.append(t)

                n_pairs = rows // 2
                odd = rows % 2
                ost = [ostp.tile([128, rows * OW], out_dt, tag=f"ost{m}",
                                 name=f"ost{m}_{y0}")
                       for m in range(2)]

                def mm_group(j, r_cnt):
                    # output rows y0+j*2 .. +r_cnt, N = r_cnt*209
                    n_free = r_cnt * OW
                    for m, mc in enumerate(M_CHUNKS):
                        ps = psp.tile([128, 512], mybir.dt.float32, tag="ps")
                        i = 0
                        for c in range(C):
                            for s2 in range(2):
                                rap = rhs[c][:, :]
                                rap.ap = mybir.VecI64Pair(
                                    [[wfree, 128], [W, r_cnt], [1, OW]])
                                rap.offset = 2 * j * W + 8 * s2
                                off = 0 if mc == 128 else 128
                                nc.tensor.matmul(
                                    ps[0:mc, 0:n_free],
                                    w_sb[:, c * 2 + s2, off:off + mc],
                                    rap,
                                    start=(i == 0), stop=(i == 5),
                                )
                                i += 1
                        nc.vector.tensor_copy(
                            ost[m][0:mc, 2 * j * OW: 2 * j * OW + n_free],
                            ps[0:mc, 0:n_free])

                for j in range(n_pairs):
                    mm_group(j, 2)
                if odd:
                    mm_group(n_pairs, 1)

                for m, mc in enumerate(M_CHUNKS):
                    dst = out_d[:, :]
                    dst.ap = mybir.VecI64Pair(
                        [[OH * OW, mc], [1, rows * OW]])
                    dst.offset = (0 if mc == 128 else 128) * (OH * OW) + y0 * OW
                    eng = nc.sync if m == 0 else nc.scalar
                    eng.dma_start(dst, ost[m][0:mc, :])

            # small leading blocks so the PE starts as early as possible
            blocks = [2, 6] + [R_BLOCK] * 33 + [3]
            assert sum(blocks) == OH
            if max_blocks is not None:
                blocks = blocks[:max_blocks]
            y0 = 0
            for rows in blocks:
                do_block(y0, rows)
                y0 += rows

    nc.compile()
    return nc


def _extract_w_tiles(x, dt):
    """Host relayout: image patches -> lhsT tiles [6, 128, 196].

    tile t = c*2 + s holds, at partition p = dy*8 + dxl, filter column n:
      w[n, c, dy, dxl + 8*s] = x[c, 16*(n//14) + dy, 16*(n%14) + dxl + 8*s]
    """
    # patches[n, c, dy, dx]
    z = x.reshape(C, NPR, P, NPR, P)           # c, pi, dy, pj, dx
    pat = z.transpose(1, 3, 0, 2, 4).reshape(NP, C, P, P)
    wt = np.empty((6, 128, NP), dtype=dt)
    for c in range(C):
        for s in range(2):
            # [16 dy, 8 dxl, n]
            blk = pat[:, c, :, 8 * s:8 * s + 8].transpose(1, 2, 0)
            wt[c * 2 + s] = blk.reshape(128, NP).astype(dt)
    return wt


_prog_cache = {}


def kernel(x: np.ndarray) -> np.ndarray:
    import os
    import sys
    # The bass program runs through jax/PJRT on the neuron (axon) backend;
    # a leftover JAX_PLATFORMS=cpu pin (commonly used to run jax references
    # on host) would hide the device. Clear it if jax isn't loaded yet.
    if os.environ.get("JAX_PLATFORMS", "") == "cpu" and "jax" not in sys.modules:
        os.environ.pop("JAX_PLATFORMS")

    from concourse.bass_utils import run_bass_kernel_spmd

    assert x.shape == (BATCH, C, H, W)
    x = np.ascontiguousarray(x, dtype=np.float32)

    dt_np = np.float16
    key = "prog"
    if key not in _prog_cache:
        _prog_cache[key] = _build_program()
    nc = _prog_cache[key]

    in_maps = []
    for b in range(BATCH):
        in_maps.append({
            "xh": x[b].astype(dt_np),
            "w": _extract_w_tiles(x[b], dt_np),
        })
    res = run_bass_kernel_spmd(nc, in_maps, list(range(N_CORES)))
    out = np.empty((BATCH, NP, OH, OW), dtype=np.float32)
    for b in range(BATCH):
        out[b] = res.results[b]["out"].astype(np.float32).reshape(NP, OH, OW)
    return out


# revision 40
# speedup vs baseline: 1.0038x; 1.0038x over previous
"""PatchSelfConv Trainium2 kernel.

Per-sample dynamic conv: each image (3,224,224) is cross-correlated
(stride 1, VALID) with its own 196 unfolded 16x16 patches as filters
-> out (196, 209, 209) per image. Batch 8 -> data-parallel, 1 image
per NeuronCore.

Per core this is out[n, y, x] = sum_{c,dy,dx} w[n,c,dy,dx] * img[c,y+dy,x+dx]
 = a [196, 768] @ [768, 209*209] matmul fed by an implicit im2col.

Implementation notes:
 - Contraction k=(c,dy,dx) is split as (c, dy, dxl+8*s): partitions hold
   (dy, dxl) with dxl in [0,8), and the remaining dx shift s in {0,1} is
   folded into the rhs access-pattern offset (+8s columns). This gives
   6 accumulating matmuls (3 c-tiles x 2 shifts) of K=128 per output
   row-pair per M-chunk, while the materialized rhs only replicates
   rows 8x instead of 16x.
 - rhs tiles [128=(dy,dxl), R*224] are built by a single 3-dim
   overlapping-window DMA per channel straight from the DRAM image
   (pre-cast to fp16 on host). Row windows are stored at width 224
   (full image rows, shifted by dxl) so (h,w) merge into one
   contiguous AP dim; matmuls read [2x209] windows at offset
   h*224 + 8*s + x.
 - Weights (the image's own patches, [768, 196] transposed-for-lhsT)
   are extracted on host (pure relayout of the input) and passed as a
   second per-core input tensor.
 - M=196 filters -> 2 chunks (128, 68). N=418 (2 output rows) per PSUM
   bank. PSUM -> SBUF via DVE cast-copy -> fp16 DMA to DRAM. Small
   6-row blocks keep the PE 100% dense (finer DMA/compute overlap).
"""
import numpy as np

C = 3
H = W = 224
P = 16
NPR = 14          # patches per row
NP = NPR * NPR    # 196
OH = OW = H - P + 1   # 209
BATCH = 8
N_CORES = 8

R_BLOCK = 6      # output rows per block
M_CHUNKS = (128, 68)


def _build_program(dt_name="float16", out_dt_name="float16", max_blocks=None):
    import concourse.bass as bass
    import concourse.tile as tile
    from concourse import bacc, mybir

    dt = getattr(mybir.dt, dt_name)
    out_dt = getattr(mybir.dt, out_dt_name)

    nc = bacc.Bacc("TRN2", target_bir_lowering=False, debug=False)
    x_d = nc.dram_tensor("xh", [C, H, W], dt, kind="ExternalInput")
    w_d = nc.dram_tensor("w", [6, 128, NP], dt, kind="ExternalInput")
    out_d = nc.dram_tensor("out", [NP, OH * OW], out_dt, kind="ExternalOutput")

    n_full = OH // R_BLOCK
    tail_rows = OH - n_full * R_BLOCK

    with tile.TileContext(nc) as tc:
        with (
            tc.tile_pool(name="wp", bufs=1) as wp,
            tc.tile_pool(name="rhs", bufs=4) as rhsp,
            tc.tile_pool(name="ost", bufs=4) as ostp,
            tc.tile_pool(name="ps", bufs=8, space="PSUM") as psp,
        ):
            # weights: [128, 6, 196], t = c*2 + s
            w_sb = wp.tile([128, 6, NP], dt)
            src = w_d[:, :, :]
            src.ap = mybir.VecI64Pair([[NP, 128], [128 * NP, 6], [1, NP]])
            src.offset = 0
            nc.scalar.dma_start(w_sb[:], src)


            def do_block(y0, rows):
                # rhs tiles per channel: [128=(dy*8+dxl), rows*224]
                wfree = rows * W
                rhs = []
                # last 7 window elements are never read by any matmul
                # (max read = (rows-1)*W + 8 + 208 = wfree - 8); skipping
                # them keeps the final block's reads inside the image.
                wdma = wfree - 7
                for c in range(C):
                    t = rhsp.tile([128, wfree], dt, tag=f"rhs{c}")
                    s = x_d[:, :, :]
                    s.ap = mybir.VecI64Pair([[W, 16], [1, 8], [1, wdma]])
                    s.offset = c * H * W + y0 * W
                    eng = nc.sync if c != 1 else nc.scalar
                    eng.dma_start(t[:, 0:wdma], s)
                    rhs.append(t)

                n_pairs = rows // 2
                odd = rows % 2

                def rhs_ap(c, j, r_cnt, s2):
                    rap = rhs[c][:, :]
                    rap.ap = mybir.VecI64Pair(
                        [[wfree, 128], [W, r_cnt], [1, OW]])
                    rap.offset = 2 * j * W + 8 * s2
                    return rap

                if rows % 8 == 0 and rows > 0:
                    # ---- col-tiled path: 4-pair groups ----
                    # chunk0: filters 0..127, serial per pair
                    ost0 = ostp.tile([128, rows * OW], out_dt, tag="ost0",
                                     name=f"ost0_{y0}")
                    for j in range(n_pairs):
                        ps = psp.tile([128, 512], mybir.dt.float32, tag="ps")
                        i = 0
                        for c in range(C):
                            for s2 in range(2):
                                nc.tensor.matmul(
                                    ps[0:128, 0:418],
                                    w_sb[:, c * 2 + s2, 0:128],
                                    rhs_ap(c, j, 2, s2),
                                    start=(i == 0), stop=(i == 5))
                                i += 1
                        nc.vector.tensor_copy(
                            ost0[0:128, 2 * j * OW: (2 * j + 2) * OW],
                            ps[0:128, 0:418])
                    dst = out_d[:, :]
                    dst.ap = mybir.VecI64Pair([[OH * OW, 128], [1, rows * OW]])
                    dst.offset = y0 * OW
                    nc.sync.dma_start(dst, ost0[:, :])

                    # chunk1: filters 128..191 (M=64), two pairs col-tiled
                    # concurrently at col groups 0 and 64
                    nsets = n_pairs // 2
                    ost1 = ostp.tile([128, nsets * 418], out_dt, tag="ost1",
                                     name=f"ost1_{y0}")
                    for st in range(nsets):
                        ps_a = psp.tile([128, 512], mybir.dt.float32, tag="ps")
                        ps_b = psp.tile([128, 512], mybir.dt.float32, tag="ps")
                        for i in range(6):
                            c, s2 = divmod(i, 2)
                            nc.tensor.matmul(
                                ps_a[0:64, 0:418],
                                w_sb[:, c * 2 + s2, 128:192],
                                rhs_ap(c, 2 * st, 2, s2),
                                start=(i == 0), stop=(i == 5),
                                tile_position=(0, 0))
                            nc.tensor.matmul(
                                ps_b[64:128, 0:418],
                                w_sb[:, c * 2 + s2, 128:192],
                                rhs_ap(c, 2 * st + 1, 2, s2),
                                start=(i == 0), stop=(i == 5),
                                tile_position=(0, 64))
                        nc.vector.tensor_copy(
                            ost1[0:64, st * 418:(st + 1) * 418],
                            ps_a[0:64, 0:418])
                        nc.vector.tensor_copy(
                            ost1[64:128, st * 418:(st + 1) * 418],
                            ps_b[64:128, 0:418])
                    for half in range(2):
                        dst = out_d[:, :]
                        dst.ap = mybir.VecI64Pair(
                            [[OH * OW, 64], [836, nsets], [1, 418]])
                        dst.offset = 128 * OH * OW + (y0 + 2 * half) * OW
                        src = ost1[:, :]
                        src.ap = mybir.VecI64Pair(
                            [[nsets * 418, 64], [418, nsets], [1, 418]])
                        src.offset = half * 64 * (nsets * 418)
                        eng = nc.sync if half == 0 else nc.scalar
                        eng.dma_start(dst, src)

                    # quad: filters 192..195 (M=4), four pairs col-tiled at
                    # col groups 0/32/64/96; results land in the persistent
                    # qost tile (partition 32q+f holds filter 192+f for
                    # rows == 2q,2q+1 mod 8)
                    for qg in range(n_pairs // 4):
                        psq = [psp.tile([128, 512], mybir.dt.float32,
                                        tag="ps", name=f"psq{q}_{y0}_{qg}")
                               for q in range(4)]
                        for i in range(6):
                            c, s2 = divmod(i, 2)
                            for q in range(4):
                                nc.tensor.matmul(
                                    psq[q][32 * q:32 * q + 4, 0:418],
                                    w_sb[:, c * 2 + s2, 192:196],
                                    rhs_ap(c, 4 * qg + q, 2, s2),
                                    start=(i == 0), stop=(i == 5),
                                    tile_position=(0, 32 * q))
                        for q in range(4):
                            nc.vector.tensor_copy(
                                qost[32 * q:32 * q + 4,
                                     (y0 + 8 * qg + 2 * q) * OW:
                                     (y0 + 8 * qg + 2 * q + 2) * OW],
                                psq[q][32 * q:32 * q + 4, 0:418])
                    return

                # ---- serial fallback path (tail rows) ----
                ost = [ostp.tile([128, rows * OW], out_dt, tag=f"ostt{m}",
                                 name=f"ostt{m}_{y0}")
                       for m in range(2)]

                def mm_group(j, r_cnt):
                    n_free = r_cnt * OW
                    for m, mc in enumerate(M_CHUNKS):
                        ps = psp.tile([128, 512], mybir.dt.float32, tag="ps")
                        i = 0
                        for c in range(C):
                            for s2 in range(2):
                                off = 0 if mc == 128 else 128
                                nc.tensor.matmul(
                                    ps[0:mc, 0:n_free],
                                    w_sb[:, c * 2 + s2, off:off + mc],
                                    rhs_ap(c, j, r_cnt, s2),
                                    start=(i == 0), stop=(i == 5),
                                )
                                i += 1
                        nc.vector.tensor_copy(
                            ost[m][0:mc, 2 * j * OW: 2 * j * OW + n_free],
                            ps[0:mc, 0:n_free])

                for j in range(n_pairs):
                    mm_group(j, 2)
                if odd:
                    mm_group(n_pairs, 1)

                for m, mc in enumerate(M_CHUNKS):
                    dst = out_d[:, :]
                    dst.ap = mybir.VecI64Pair(
                        [[OH * OW, mc], [1, rows * OW]])
                    dst.offset = (0 if mc == 128 else 128) * (OH * OW) + y0 * OW
                    eng = nc.sync if m == 0 else nc.scalar
                    eng.dma_start(dst, ost[m][0:mc, :])

            qost = wp.tile([128, (OH - 1) * OW], out_dt, name="qost")

            blocks = [8] * 26 + [1]
            assert sum(blocks) == OH
            if max_blocks is not None:
                blocks = blocks[:max_blocks]
            y0 = 0
            for rows in blocks:
                do_block(y0, rows)
                y0 += rows

            # filters 192..195 for rows 0..207, from the persistent qost
            QFS = (OH - 1) * OW
            for q in range(4):
                src = qost[:, :]
                src.ap = mybir.VecI64Pair(
                    [[QFS, 4], [8 * OW, 26], [1, 418]])
                src.offset = (32 * q) * QFS + 2 * q * OW
                dst = out_d[:, :]
                dst.ap = mybir.VecI64Pair(
                    [[OH * OW, 4], [8 * OW, 26], [1, 418]])
                dst.offset = 192 * OH * OW + 2 * q * OW
                eng = nc.sync if q % 2 == 0 else nc.scalar
                eng.dma_start(dst, src)

    nc.compile()
    return nc


def _extract_w_tiles(x, dt):
    """Host relayout: image patches -> lhsT tiles [6, 128, 196].

    tile t = c*2 + s holds, at partition p = dy*8 + dxl, filter column n:
      w[n, c, dy, dxl + 8*s] = x[c, 16*(n//14) + dy, 16*(n%14) + dxl + 8*s]
    """
    # patches[n, c, dy, dx]
    z = x.reshape(C, NPR, P, NPR, P)           # c, pi, dy, pj, dx
    pat = z.transpose(1, 3, 0, 2, 4).reshape(NP, C, P, P)
    wt = np.empty((6, 128, NP), dtype=dt)
    for c in range(C):
        for s in range(2):
            # [16 dy, 8 dxl, n]
            blk = pat[:, c, :, 8 * s:8 * s + 8].transpose(1, 2, 0)
            wt[c * 2 + s] = blk.reshape(128, NP).astype(dt)
    return wt


_prog_cache = {}


def kernel(x: np.ndarray) -> np.ndarray:
    import os
    import sys
    # The bass program runs through jax/PJRT on the neuron (axon) backend;
    # a leftover JAX_PLATFORMS=cpu pin (commonly used to run jax references
    # on host) would hide the device. Clear it if jax isn't loaded yet.
    if os.environ.get("JAX_PLATFORMS", "") == "cpu" and "jax" not in sys.modules:
        os.environ.pop("JAX_PLATFORMS")

    from concourse.bass_utils import run_bass_kernel_spmd

    assert x.shape == (BATCH, C, H, W)
    x = np.ascontiguousarray(x, dtype=np.float32)

    dt_np = np.float16
    key = "prog"
    if key not in _prog_cache:
        _prog_cache[key] = _build_program()
    nc = _prog_cache[key]

    in_maps = []
    for b in range(BATCH):
        in_maps.append({
            "xh": x[b].astype(dt_np),
            "w": _extract_w_tiles(x[b], dt_np),
        })
    res = run_bass_kernel_spmd(nc, in_maps, list(range(N_CORES)))
    out = np.empty((BATCH, NP, OH, OW), dtype=np.float32)
    for b in range(BATCH):
        out[b] = res.results[b]["out"].astype(np.float32).reshape(NP, OH, OW)
    return out


# revision 41
# speedup vs baseline: 1.0732x; 1.0692x over previous
"""PatchSelfConv Trainium2 kernel.

Per-sample dynamic conv: each image (3,224,224) is cross-correlated
(stride 1, VALID) with its own 196 unfolded 16x16 patches as filters
-> out (196, 209, 209) per image. Batch 8 -> data-parallel, 1 image
per NeuronCore.

Per core this is out[n, y, x] = sum_{c,dy,dx} w[n,c,dy,dx] * img[c,y+dy,x+dx]
 = a [196, 768] @ [768, 209*209] matmul fed by an implicit im2col.

Implementation notes:
 - Contraction k=(c,dy,dx) is split as (c, dy, dxl+8*s): partitions hold
   (dy, dxl) with dxl in [0,8), and the remaining dx shift s in {0,1} is
   folded into the rhs access-pattern offset (+8s columns). This gives
   6 accumulating matmuls (3 c-tiles x 2 shifts) of K=128 per output
   row-pair per M-chunk, while the materialized rhs only replicates
   rows 8x instead of 16x.
 - rhs tiles [128=(dy,dxl), R*224] are built by a single 3-dim
   overlapping-window DMA per channel straight from the DRAM image
   (pre-cast to fp16 on host). Row windows are stored at width 224
   (full image rows, shifted by dxl) so (h,w) merge into one
   contiguous AP dim; matmuls read [2x209] windows at offset
   h*224 + 8*s + x.
 - Weights (the image's own patches, [768, 196] transposed-for-lhsT)
   are extracted on host (pure relayout of the input) and passed as a
   second per-core input tensor.
 - M=196 filters -> 2 chunks (128, 68). N=418 (2 output rows) per PSUM
   bank. PSUM -> SBUF via DVE cast-copy -> fp16 DMA to DRAM. Small
   6-row blocks keep the PE 100% dense (finer DMA/compute overlap).
"""
import numpy as np

C = 3
H = W = 224
P = 16
NPR = 14          # patches per row
NP = NPR * NPR    # 196
OH = OW = H - P + 1   # 209
BATCH = 8
N_CORES = 8

R_BLOCK = 6      # output rows per block
M_CHUNKS = (128, 68)


def _build_program(dt_name="float16", out_dt_name="float16", max_blocks=None):
    import concourse.bass as bass
    import concourse.tile as tile
    from concourse import bacc, mybir

    dt = getattr(mybir.dt, dt_name)
    out_dt = getattr(mybir.dt, out_dt_name)

    nc = bacc.Bacc("TRN2", target_bir_lowering=False, debug=False)
    x_d = nc.dram_tensor("xh", [C, H, W], dt, kind="ExternalInput")
    w_d = nc.dram_tensor("w", [6, 128, NP], dt, kind="ExternalInput")
    out_d = nc.dram_tensor("out", [NP, OH * OW], out_dt, kind="ExternalOutput")

    n_full = OH // R_BLOCK
    tail_rows = OH - n_full * R_BLOCK

    with tile.TileContext(nc) as tc:
        with (
            tc.tile_pool(name="wp", bufs=1) as wp,
            tc.tile_pool(name="rhs", bufs=4) as rhsp,
            tc.tile_pool(name="ost", bufs=4) as ostp,
            tc.tile_pool(name="ps", bufs=8, space="PSUM") as psp,
        ):
            # weights: [128, 6, 196], t = c*2 + s
            w_sb = wp.tile([128, 6, NP], dt)
            src = w_d[:, :, :]
            src.ap = mybir.VecI64Pair([[NP, 128], [128 * NP, 6], [1, NP]])
            src.offset = 0
            nc.scalar.dma_start(w_sb[:], src)


            def do_block(y0, rows):
                # rhs tiles per channel: [128=(dy*8+dxl), rows*224]
                wfree = rows * W
                rhs = []
                # last 7 window elements are never read by any matmul
                # (max read = (rows-1)*W + 8 + 208 = wfree - 8); skipping
                # them keeps the final block's reads inside the image.
                wdma = wfree - 7
                for c in range(C):
                    t = rhsp.tile([128, wfree], dt, tag=f"rhs{c}")
                    s = x_d[:, :, :]
                    s.ap = mybir.VecI64Pair([[W, 16], [1, 8], [1, wdma]])
                    s.offset = c * H * W + y0 * W
                    eng = nc.sync if c != 1 else nc.scalar
                    eng.dma_start(t[:, 0:wdma], s)
                    rhs.append(t)

                n_pairs = rows // 2
                odd = rows % 2

                def rhs_ap(c, j, r_cnt, s2):
                    rap = rhs[c][:, :]
                    rap.ap = mybir.VecI64Pair(
                        [[wfree, 128], [W, r_cnt], [1, OW]])
                    rap.offset = 2 * j * W + 8 * s2
                    return rap

                if rows % 8 == 0 and rows > 0:
                    # ---- col-tiled path: 4-pair groups ----
                    # chunk0: filters 0..127, serial per pair
                    ost0 = ostp.tile([128, rows * OW], out_dt, tag="ost0",
                                     name=f"ost0_{y0}")
                    for j in range(n_pairs):
                        ps = psp.tile([128, 512], mybir.dt.float32, tag="ps")
                        i = 0
                        for c in range(C):
                            for s2 in range(2):
                                nc.tensor.matmul(
                                    ps[0:128, 0:418],
                                    w_sb[:, c * 2 + s2, 0:128],
                                    rhs_ap(c, j, 2, s2),
                                    start=(i == 0), stop=(i == 5))
                                i += 1
                        nc.vector.tensor_copy(
                            ost0[0:128, 2 * j * OW: (2 * j + 2) * OW],
                            ps[0:128, 0:418])
                    dst = out_d[:, :]
                    dst.ap = mybir.VecI64Pair([[OH * OW, 128], [1, rows * OW]])
                    dst.offset = y0 * OW
                    nc.sync.dma_start(dst, ost0[:, :])

                    # chunk1: filters 128..191 (M=64), two pairs col-tiled
                    # concurrently at col groups 0 and 64
                    nsets = n_pairs // 2
                    ost1 = ostp.tile([128, nsets * 418], out_dt, tag="ost1",
                                     name=f"ost1_{y0}")
                    for st in range(nsets):
                        ps_a = psp.tile([128, 512], mybir.dt.float32, tag="ps")
                        ps_b = psp.tile([128, 512], mybir.dt.float32, tag="ps")
                        for i in range(6):
                            c, s2 = divmod(i, 2)
                            nc.tensor.matmul(
                                ps_a[0:64, 0:418],
                                w_sb[:, c * 2 + s2, 128:192],
                                rhs_ap(c, 2 * st, 2, s2),
                                start=(i == 0), stop=(i == 5),
                                tile_position=(0, 0))
                            nc.tensor.matmul(
                                ps_b[64:128, 0:418],
                                w_sb[:, c * 2 + s2, 128:192],
                                rhs_ap(c, 2 * st + 1, 2, s2),
                                start=(i == 0), stop=(i == 5),
                                tile_position=(0, 64))
                        nc.vector.tensor_copy(
                            ost1[0:64, st * 418:(st + 1) * 418],
                            ps_a[0:64, 0:418])
                        nc.vector.tensor_copy(
                            ost1[64:128, st * 418:(st + 1) * 418],
                            ps_b[64:128, 0:418])
                    for half in range(2):
                        dst = out_d[:, :]
                        dst.ap = mybir.VecI64Pair(
                            [[OH * OW, 64], [836, nsets], [1, 418]])
                        dst.offset = 128 * OH * OW + (y0 + 2 * half) * OW
                        src = ost1[:, :]
                        src.ap = mybir.VecI64Pair(
                            [[nsets * 418, 64], [418, nsets], [1, 418]])
                        src.offset = half * 64 * (nsets * 418)
                        eng = nc.sync if half == 0 else nc.scalar
                        eng.dma_start(dst, src)

                    # quad: filters 192..195 (M=4), four pairs col-tiled at
                    # col groups 0/32/64/96; results land in the persistent
                    # qost tile (partition 32q+f holds filter 192+f for
                    # rows == 2q,2q+1 mod 8)
                    for qg in range(n_pairs // 4):
                        psq = [psp.tile([128, 512], mybir.dt.float32,
                                        tag="ps", name=f"psq{q}_{y0}_{qg}")
                               for q in range(4)]
                        for i in range(6):
                            c, s2 = divmod(i, 2)
                            for q in range(4):
                                nc.tensor.matmul(
                                    psq[q][32 * q:32 * q + 4, 0:418],
                                    w_sb[:, c * 2 + s2, 192:196],
                                    rhs_ap(c, 4 * qg + q, 2, s2),
                                    start=(i == 0), stop=(i == 5),
                                    tile_position=(0, 32 * q))
                        for q in range(4):
                            nc.vector.tensor_copy(
                                qost[32 * q:32 * q + 4,
                                     (y0 + 8 * qg + 2 * q) * OW:
                                     (y0 + 8 * qg + 2 * q + 2) * OW],
                                psq[q][32 * q:32 * q + 4, 0:418])
                    return

                # ---- serial fallback path (tail rows) ----
                ost = [ostp.tile([128, rows * OW], out_dt, tag=f"ostt{m}",
                                 name=f"ostt{m}_{y0}")
                       for m in range(2)]

                def mm_group(j, r_cnt):
                    n_free = r_cnt * OW
                    for m, mc in enumerate(M_CHUNKS):
                        ps = psp.tile([128, 512], mybir.dt.float32, tag="ps")
                        i = 0
                        for c in range(C):
                            for s2 in range(2):
                                off = 0 if mc == 128 else 128
                                nc.tensor.matmul(
                                    ps[0:mc, 0:n_free],
                                    w_sb[:, c * 2 + s2, off:off + mc],
                                    rhs_ap(c, j, r_cnt, s2),
                                    start=(i == 0), stop=(i == 5),
                                )
                                i += 1
                        nc.vector.tensor_copy(
                            ost[m][0:mc, 2 * j * OW: 2 * j * OW + n_free],
                            ps[0:mc, 0:n_free])

                for j in range(n_pairs):
                    mm_group(j, 2)
                if odd:
                    mm_group(n_pairs, 1)

                for m, mc in enumerate(M_CHUNKS):
                    dst = out_d[:, :]
                    dst.ap = mybir.VecI64Pair(
                        [[OH * OW, mc], [1, rows * OW]])
                    dst.offset = (0 if mc == 128 else 128) * (OH * OW) + y0 * OW
                    eng = nc.sync if m == 0 else nc.scalar
                    eng.dma_start(dst, ost[m][0:mc, :])

            qost = wp.tile([128, (OH - 1) * OW], out_dt, name="qost")
            QFS = (OH - 1) * OW

            def quad_flush(row0, nblk):
                # filters 192..195 for the col-tiled blocks starting at row0
                for q in range(4):
                    src = qost[:, :]
                    src.ap = mybir.VecI64Pair(
                        [[QFS, 4], [8 * OW, nblk], [1, 418]])
                    src.offset = (32 * q) * QFS + (row0 + 2 * q) * OW
                    dst = out_d[:, :]
                    dst.ap = mybir.VecI64Pair(
                        [[OH * OW, 4], [8 * OW, nblk], [1, 418]])
                    dst.offset = 192 * OH * OW + (row0 + 2 * q) * OW
                    eng = nc.sync if q % 2 == 0 else nc.scalar
                    eng.dma_start(dst, src)

            blocks = [2, 6] + [8] * 25 + [1]
            assert sum(blocks) == OH
            if max_blocks is not None:
                blocks = blocks[:max_blocks]
            y0 = 0
            flushed = False
            for rows in blocks:
                do_block(y0, rows)
                y0 += rows
                if y0 == 168 and not flushed:
                    quad_flush(8, 20)   # col-tiled blocks at rows 8..167
                    flushed = True
            if max_blocks is None:
                quad_flush(168, 5)      # rows 168..207

    nc.compile()
    return nc


def _extract_w_tiles(x, dt):
    """Host relayout: image patches -> lhsT tiles [6, 128, 196].

    tile t = c*2 + s holds, at partition p = dy*8 + dxl, filter column n:
      w[n, c, dy, dxl + 8*s] = x[c, 16*(n//14) + dy, 16*(n%14) + dxl + 8*s]
    """
    # patches[n, c, dy, dx]
    z = x.reshape(C, NPR, P, NPR, P)           # c, pi, dy, pj, dx
    pat = z.transpose(1, 3, 0, 2, 4).reshape(NP, C, P, P)
    wt = np.empty((6, 128, NP), dtype=dt)
    for c in range(C):
        for s in range(2):
            # [16 dy, 8 dxl, n]
            blk = pat[:, c, :, 8 * s:8 * s + 8].transpose(1, 2, 0)
            wt[c * 2 + s] = blk.reshape(128, NP).astype(dt)
    return wt


_prog_cache = {}


def kernel(x: np.ndarray) -> np.ndarray:
    import os
    import sys
    # The bass program runs through jax/PJRT on the neuron (axon) backend;
    # a leftover JAX_PLATFORMS=cpu pin (commonly used to run jax references
    # on host) would hide the device. Clear it if jax isn't loaded yet.
    if os.environ.get("JAX_PLATFORMS", "") == "cpu" and "jax" not in sys.modules:
        os.environ.pop("JAX_PLATFORMS")

    from concourse.bass_utils import run_bass_kernel_spmd

    assert x.shape == (BATCH, C, H, W)
    x = np.ascontiguousarray(x, dtype=np.float32)

    dt_np = np.float16
    key = "prog"
    if key not in _prog_cache:
        _prog_cache[key] = _build_program()
    nc = _prog_cache[key]

    in_maps = []
    for b in range(BATCH):
        in_maps.append({
            "xh": x[b].astype(dt_np),
            "w": _extract_w_tiles(x[b], dt_np),
        })
    res = run_bass_kernel_spmd(nc, in_maps, list(range(N_CORES)))
    out = np.empty((BATCH, NP, OH, OW), dtype=np.float32)
    for b in range(BATCH):
        out[b] = res.results[b]["out"].astype(np.float32).reshape(NP, OH, OW)
    return out
